# revision 26
# baseline (speedup 1.0000x reference)
"""Bass/Trainium2 kernel for nn_Head_13030930776875.

out = 0.7*softmax(causal(x@Wq @ (x@Wk)^T / sqrt(d))) @ (x@Wv)
    + 0.3*rownorm(causal(exp(-|y_i - y_j|^2 / (2d)))) @ (x@Wv),  y = (x@Wk)@L_grav

Sharding: 8 cores = 4 samples x 2 halves. Each half owns two 512-row query
groups chosen so causal (triangular) work balances: half0 -> {G0, G3},
half1 -> {G1, G2}.

Permuted key layout: the host ships x^T with its 512-column groups permuted
per half (h0: [G0,G1,G3,G2], h1: [G1,G0,G2,G3]) so that SBUF group 0 is the
pos0 query group and group 2 is the pos1 query group FOR EVERY CORE. The
program is SPMD-uniform; causality over the permuted key order is data-
driven via a tiny per-pair exp-bias table (0 / -40 kills whole chunks) plus
compile-time affine_select band masks for the diagonal chunks (which sit at
the same slot positions on every core).

On-device layout: everything transposed (d on partitions). Scores are
computed as s^T tiles [k,q] so that (a) A^T slices feed the A@v matmul
directly as the stationary operand (no transposes anywhere), and (b) the
causal row-sums come free via ones-columns appended to v.

Score exps are batched: two 512-wide score matmuls share one [128,1024]
PSUM tile and a single exp ACT whose per-partition bias AP kills fully-
invalid chunk pairs. The grav kernel factorizes as exp(gram/128)*ek with
ek = exp(-sq_k/256) folded into a second v copy (vg = v*ek) so no per-slot
mask multiplies are needed anywhere; diagonal bands are zeroed in-place by
gpsimd affine_select.
"""

import math
import os

import numpy as np

B, N, D_MODEL, D_HEAD = 4, 2048, 1024, 128
OMEGA_LANG, OMEGA_GRAV = 0.7, 0.3
SC_LANG = 1.0 / math.sqrt(D_HEAD)
SC_GRAV = 1.0 / D_HEAD
NBLK = N // 128            # 16 k-chunks of 128
NCH = (8, 16)              # chunks per position (pos0 group, pos1 group)
NSLOT = 16                 # mask slots: pos0 loop 0-7, pos1 loop 8-15
KILL = -40.0               # exp bias for fully-invalid chunk pairs
# chunk processing order per position (band group last so the A@v prefix
# structure nkb = 5+j / 13+j covers exactly the causally-needed chunks)
P0 = [4, 5, 6, 7, 0, 1, 2, 3]
P1 = [0, 1, 2, 3, 4, 5, 6, 7, 12, 13, 14, 15, 8, 9, 10, 11]
# original group id per SBUF group slot, per half
GROUPS = [[0, 1, 3, 2], [1, 0, 2, 3]]

_CACHE = {}


def _build_nc():
    import concourse.bacc as bacc
    import concourse.mybir as mybir
    import concourse.tile as tile
    import concourse.bass as bass

    dt = mybir.dt
    F16, F32 = dt.float16, dt.float32
    AF = mybir.ActivationFunctionType
    OP = mybir.AluOpType

    nc = bacc.Bacc()

    xT = nc.declare_dram_parameter("xT", [D_MODEL, N], F16, isOutput=False)
    wq = nc.declare_dram_parameter("wq", [128, 8 * 128], F16, isOutput=False)
    wk = nc.declare_dram_parameter("wk", [128, 8 * 128], F16, isOutput=False)
    wv = nc.declare_dram_parameter("wv", [128, 8 * 128], F16, isOutput=False)
    lg = nc.declare_dram_parameter("lg", [128, 128], F16, isOutput=False)
    bf = nc.declare_dram_parameter("bf", [128, 12], F32, isOutput=False)
    out_d = nc.declare_dram_parameter("out", [N // 2, 128], F16, isOutput=True)

    with tile.TileContext(nc) as tc:
        with (
            tc.tile_pool(name="big", bufs=1) as big,
            tc.tile_pool(name="xtp", bufs=8) as xtp,
            tc.tile_pool(name="ap0", bufs=1) as ap0,
            tc.tile_pool(name="ap1", bufs=1) as ap1,
            tc.tile_pool(name="small", bufs=4) as small,
            tc.tile_pool(name="pols", bufs=4) as pols,
            tc.tile_pool(name="outp", bufs=4) as outp,
            tc.tile_pool(name="score", bufs=2, space="PSUM") as score,
            tc.tile_pool(name="pp", bufs=2, space="PSUM") as pp,
            tc.tile_pool(name="av", bufs=2, space="PSUM") as av,
        ):
            # ---- small inputs on the scalar DGE ring (parallel to x) ----
            wq_s = big.tile([128, 8, 128], F16, tag="wq")
            wk_s = big.tile([128, 8, 128], F16, tag="wk")
            wv_s = big.tile([128, 8, 128], F16, tag="wv")
            lg_s = big.tile([128, 128], F16, tag="lg")
            bf_s = big.tile([128, 12], F32, tag="bf")
            nc.scalar.dma_start(wk_s[:, 0, :], wk[:, 0:128])
            nc.scalar.dma_start(wk_s[:, 1:8, :],
                                wk[:, 128:1024].rearrange("p (c d) -> p c d", c=7))
            nc.scalar.dma_start(lg_s[:], lg[:])
            nc.scalar.dma_start(wq_s[:], wq[:].rearrange("p (c d) -> p c d", c=8))
            nc.scalar.dma_start(wv_s[:], wv[:].rearrange("p (c d) -> p c d", c=8))
            nc.scalar.dma_start(bf_s[:], bf[:])

            # ---- x^T in 1024-col halves (2KB DMA lines), need-ordered ----
            xt = [xtp.tile([128, N], F16, tag="xt", name=f"xt{c}")
                  for c in range(8)]
            nc.sync.dma_start(xt[0][:, 0:512], xT[0:128, 0:512])
            nc.sync.dma_start(xt[0][:, 512:1024], xT[0:128, 512:1024])
            for c in range(1, 8):
                nc.sync.dma_start(xt[c][:, 0:1024], xT[c * 128:(c + 1) * 128, 0:1024])
            for c in range(8):
                nc.sync.dma_start(xt[c][:, 1024:2048],
                                  xT[c * 128:(c + 1) * 128, 1024:2048])

            # ---- persistent intermediates ----
            kT = big.tile([128, N], F16, tag="kT")
            qT = big.tile([128, N // 2], F16, tag="qT")
            yT = big.tile([128, N], F16, tag="yT")
            sqn = big.tile([128, NBLK], F32, tag="sqn")
            ek_s = big.tile([128, NBLK], F32, tag="ek")
            vaug = big.tile([128, NBLK, 130], F16, tag="vaug")
            vg = big.tile([128, NBLK, 130], F16, tag="vg")
            nc.vector.memset(vaug[:, :, 128:129], 1.0 / OMEGA_LANG)
            nc.vector.memset(vaug[:, :, 129:130], 1.0 / OMEGA_GRAV)

            def proj_group(dst, dcol, w_sb, g):
                cols = slice(g * 512, (g + 1) * 512)
                ps = pp.tile([128, 512], F32, tag="pp")
                for c in range(8):
                    nc.tensor.matmul(ps[:], w_sb[:, c, :], xt[c][:, cols],
                                     start=(c == 0), stop=(c == 7))
                nc.vector.tensor_copy(dst[:, dcol * 512:(dcol + 1) * 512], ps[:])

            def yt_group(g):
                cols = slice(g * 512, (g + 1) * 512)
                ps = pp.tile([128, 512], F32, tag="pp")
                nc.tensor.matmul(ps[:], lg_s[:], kT[:, cols])
                nc.vector.tensor_copy(yT[:, cols], ps[:])

            def sqn_quad(q4):
                # y chunks for 4 key blocks -> [128,4,128] psum; squared on
                # ACT ((y/16)^2 = y^2/256), then one DVE reduce -> sq/256.
                ps = pp.tile([128, 4, 128], F32, tag="pp")
                for i in range(4):
                    kb = q4 * 4 + i
                    nc.tensor.matmul(ps[:, i, :], kT[:, kb * 128:(kb + 1) * 128],
                                     lg_s[:])
                scr = small.tile([128, 4, 128], F32, tag="scr")
                nc.scalar.activation(scr[:], ps[:], AF.Square, scale=0.0625)
                nc.vector.tensor_reduce(sqn[:, q4 * 4:(q4 + 1) * 4], scr[:],
                                        mybir.AxisListType.X, OP.add)

            def vaug_chunk(kb):
                ps = pp.tile([128, 512], F32, tag="pp")
                for c in range(8):
                    nc.tensor.matmul(ps[:, 0:128], xt[c][:, kb * 128:(kb + 1) * 128],
                                     wv_s[:, c, :], start=(c == 0), stop=(c == 7))
                nc.vector.tensor_copy(vaug[:, kb, 0:128], ps[:, 0:128])

            def vg_chunk(kb):
                # grav v copy with ek = exp(-sq_k/256) folded in; col 129
                # becomes ek/OMEGA_GRAV (the grav rowsum weight).
                nc.vector.tensor_scalar(vg[:, kb, :], vaug[:, kb, :],
                                        ek_s[:, kb:kb + 1], None, OP.mult)

            def diag_select(t, pos, i):
                # zero the causally-invalid band of own-group chunk i
                # (slot 4+i for pos0, 12+i for pos1): valid iff q >= p + i*128
                s = (4 if pos == 0 else 12) + i
                nc.gpsimd.affine_select(
                    t[:, s * 512:(s + 1) * 512], t[:, s * 512:(s + 1) * 512],
                    pattern=[[1, 512]], compare_op=OP.is_ge, fill=0.0,
                    base=-i * 128, channel_multiplier=-1)

            # PE warmup woven into the first kT projection group: 2 dummy
            # matmuls per x chunk, each consuming only already-arrived data,
            # so the PE tracks DMA arrival and the HAM clock-gate opens
            # (4/8 -> 8/8) by the time the dense stream begins.
            def kt_g0_with_warmup():
                warm = pp.tile([128, 512], F32, tag="pp")
                ps = pp.tile([128, 512], F32, tag="pp")
                for c in range(8):
                    for i in range(2):
                        nc.tensor.matmul(warm[:], xt[c][:, 0:128],
                                         xt[c][:, 0:512],
                                         start=(c == 0 and i == 0),
                                         stop=(c == 7 and i == 1))
                    nc.tensor.matmul(ps[:], wk_s[:, c, :], xt[c][:, 0:512],
                                     start=(c == 0), stop=(c == 7))
                nc.vector.tensor_copy(kT[:, 0:512], ps[:])

            def score_pair(dst, lhs, slot, plist, qrhs, scale, bfc):
                # two 512-wide score matmuls (chunks plist[slot], plist[slot+1])
                # -> one [128,1024] psum tile, one exp ACT whose per-partition
                # bias (0 / KILL) drops fully-invalid chunk pairs.
                ca, cb = plist[slot], plist[slot + 1]
                ps = score.tile([128, 1024], F32, tag="sc")
                nc.tensor.matmul(ps[:, 0:512], lhs[:, ca * 128:(ca + 1) * 128], qrhs)
                nc.tensor.matmul(ps[:, 512:1024], lhs[:, cb * 128:(cb + 1) * 128],
                                 qrhs)
                nc.scalar.activation(dst[:, slot * 512:(slot + 2) * 512], ps[:],
                                     AF.Exp, scale=scale,
                                     bias=bf_s[:, bfc:bfc + 1])

            def attn_j(pos, j):
                # A^T @ v_aug for one 128-row query block; rowsums ride cols
                # 128 (lang, 1/0.7) and 129 (grav, ek/0.3). The grav chain
                # accumulates chunks whose exp lands latest at the end so the
                # PE can start the chain before the last grav ACTs drain.
                nkb = (5 + j) if pos == 0 else (13 + j)
                plist = P0 if pos == 0 else P1
                ks = list(range(nkb))
                gks = ([kb for kb in ks if kb >= 12] +
                       [kb for kb in ks if kb < 8] +
                       [kb for kb in ks if 8 <= kb < 12]) if pos == 1 else ks
                pol = av.tile([128, 132], F32, tag="av")
                pog = av.tile([128, 132], F32, tag="av")
                for kb in ks:
                    nc.tensor.matmul(pol[:, 0:129],
                                     alang[pos][:, kb * 512 + j * 128:kb * 512 + (j + 1) * 128],
                                     vaug[:, plist[kb], 0:129],
                                     start=(kb == ks[0]), stop=(kb == ks[-1]))
                for kb in gks:
                    nc.tensor.matmul(pog[:, 0:130],
                                     agrav[pos][:, kb * 512 + j * 128:kb * 512 + (j + 1) * 128],
                                     vg[:, plist[kb], 0:130],
                                     start=(kb == gks[0]), stop=(kb == gks[-1]))
                blend_out(pos, j, pol, pog)

            def blend_out(pos, j, pol, pog):
                rl = small.tile([128, 1], F32, tag="rl")
                rg = small.tile([128, 1], F32, tag="rg")
                nc.vector.reciprocal(rl[:], pol[:, 128:129])
                nc.vector.reciprocal(rg[:], pog[:, 129:130])
                ob = outp.tile([128, 128], F32, tag="ob")
                ob2 = outp.tile([128, 128], F16, tag="ob2")
                nc.vector.tensor_scalar(ob[:], pol[:, 0:128], rl[:], None, OP.mult)
                nc.vector.scalar_tensor_tensor(ob2[:], pog[:, 0:128], rg[:], ob[:],
                                               OP.mult, OP.add)
                r0 = pos * 512 + j * 128
                nc.sync.dma_start(out_d[r0:r0 + 128, :], ob2[:])

            def attn_pol(pos, j):
                # lang-only A@v chain (no ACT dependency); numerator+rowsum
                # parked in SBUF so the PSUM bank recycles under the
                # score-exp phase still in flight.
                nkb = (5 + j) if pos == 0 else (13 + j)
                plist = P0 if pos == 0 else P1
                pol = av.tile([128, 132], F32, tag="av")
                for kb in range(nkb):
                    nc.tensor.matmul(pol[:, 0:129],
                                     alang[pos][:, kb * 512 + j * 128:kb * 512 + (j + 1) * 128],
                                     vaug[:, plist[kb], 0:129],
                                     start=(kb == 0), stop=(kb == nkb - 1))
                psb = pols.tile([128, 132], F32, tag="polsb")
                nc.vector.tensor_copy(psb[:, 0:129], pol[:, 0:129])
                rl = small.tile([128, 1], F32, tag="rl")
                nc.vector.reciprocal(rl[:], pol[:, 128:129])
                ob = outp.tile([128, 128], F32, tag="obp")
                nc.vector.tensor_scalar(ob[:], pol[:, 0:128], rl[:], None, OP.mult)
                return (psb, ob)

            def attn_pog(pos, j, psb_ob):
                psb, ob = psb_ob
                nkb = (5 + j) if pos == 0 else (13 + j)
                plist = P0 if pos == 0 else P1
                ks = list(range(nkb))
                gks = ([kb for kb in ks if kb >= 12] +
                       [kb for kb in ks if kb < 8] +
                       [kb for kb in ks if 8 <= kb < 12]) if pos == 1 else ks
                pog = av.tile([128, 132], F32, tag="av")
                for kb in gks:
                    nc.tensor.matmul(pog[:, 0:130],
                                     agrav[pos][:, kb * 512 + j * 128:kb * 512 + (j + 1) * 128],
                                     vg[:, plist[kb], 0:130],
                                     start=(kb == gks[0]), stop=(kb == gks[-1]))
                rg = small.tile([128, 1], F32, tag="rg")
                nc.vector.reciprocal(rg[:], pog[:, 129:130])
                ob2 = outp.tile([128, 128], F16, tag="ob2")
                nc.vector.scalar_tensor_tensor(ob2[:], pog[:, 0:128], rg[:], ob[:],
                                               OP.mult, OP.add)
                r0 = pos * 512 + j * 128
                nc.scalar.dma_start(out_d[r0:r0 + 128, :], ob2[:])

            # ================= h0 stream, tracked: kT groups 0+1 accumulate
            # chunk-by-chunk as the first x halves land. The 2 MMs/chunk at
            # cold clock (~850ns) match the ~740ns chunk arrival cadence, so
            # this doubles as the HAM warmup with zero dummy work.
            ps_k0 = pp.tile([128, 512], F32, tag="pp")
            ps_k1 = pp.tile([128, 512], F32, tag="pp")
            for c in range(8):
                nc.tensor.matmul(ps_k0[:], wk_s[:, c, :], xt[c][:, 0:512],
                                 start=(c == 0), stop=(c == 7))
                nc.tensor.matmul(ps_k1[:], wk_s[:, c, :], xt[c][:, 512:1024],
                                 start=(c == 0), stop=(c == 7))
            nc.vector.tensor_copy(kT[:, 0:512], ps_k0[:])
            nc.vector.tensor_copy(kT[:, 512:1024], ps_k1[:])
            proj_group(qT, 0, wq_s, 0)
            q0 = qT[:, 0:512]
            yq0 = yT[:, 0:512]
            yt_group(0)
            yt_group(1)

            alang = [None, None]
            agrav = [None, None]
            alang[0] = ap0.tile([128, NCH[0] * 512], F16, tag="al0", name="al0")
            agrav[0] = ap0.tile([128, NCH[0] * 512], F16, tag="ag0", name="ag0")

            # pos0 lang scores; fillers: sqn quads + the tracked kT g2 /
            # qT g1 projections that consume the h1 halves as they land
            score_pair(alang[0], kT, 0, P0, q0, SC_LANG, 0)
            sqn_quad(0)
            score_pair(alang[0], kT, 2, P0, q0, SC_LANG, 1)
            sqn_quad(1)
            score_pair(alang[0], kT, 4, P0, q0, SC_LANG, 2)
            nc.scalar.activation(ek_s[:, 0:8], sqn[:, 0:8], AF.Exp, scale=-1.0)
            diag_select(alang[0], 0, 0)
            diag_select(alang[0], 0, 1)
            score_pair(alang[0], kT, 6, P0, q0, SC_LANG, 3)
            diag_select(alang[0], 0, 2)
            diag_select(alang[0], 0, 3)

            # pos0 grav scores; fillers: vaug chunks 0-7 (+ vg) and the
            # remaining tracked h1 projections
            for p in range(4):
                score_pair(agrav[0], yT, 2 * p, P0, yq0, SC_GRAV, p)
                if p >= 2:
                    diag_select(agrav[0], 0, 2 * p - 4)
                    diag_select(agrav[0], 0, 2 * p - 3)
                vaug_chunk(2 * p)
                vg_chunk(2 * p)
                vaug_chunk(2 * p + 1)
                vg_chunk(2 * p + 1)
            # by now all second x halves have landed: qT g1 runs stall-free
            proj_group(qT, 1, wq_s, 2)
            q1 = qT[:, 512:1024]
            yq1 = yT[:, 1024:1536]

            # ================= pos1 lang scores; fillers: pos0 A@v chains,
            # kT g3, yt/sqn for the upper groups, vaug chunks 8-15
            alang[1] = ap1.tile([128, NCH[1] * 512], F16, tag="al1", name="al1")
            agrav[1] = ap1.tile([128, NCH[1] * 512], F16, tag="ag1", name="ag1")
            score_pair(alang[1], kT, 0, P1, q1, SC_LANG, 4)
            attn_j(0, 0)
            score_pair(alang[1], kT, 2, P1, q1, SC_LANG, 5)
            attn_j(0, 1)
            score_pair(alang[1], kT, 4, P1, q1, SC_LANG, 6)
            proj_group(kT, 3, wk_s, 3)
            score_pair(alang[1], kT, 6, P1, q1, SC_LANG, 7)
            proj_group(kT, 2, wk_s, 2)
            score_pair(alang[1], kT, 8, P1, q1, SC_LANG, 8)
            yt_group(2)
            sqn_quad(2)
            score_pair(alang[1], kT, 10, P1, q1, SC_LANG, 9)
            yt_group(3)
            sqn_quad(3)
            nc.scalar.activation(ek_s[:, 8:16], sqn[:, 8:16], AF.Exp,
                                 scale=-1.0)
            score_pair(alang[1], kT, 12, P1, q1, SC_LANG, 10)
            diag_select(alang[1], 1, 0)
            diag_select(alang[1], 1, 1)
            vaug_chunk(8)
            vg_chunk(8)
            vaug_chunk(9)
            vg_chunk(9)
            score_pair(alang[1], kT, 14, P1, q1, SC_LANG, 11)
            diag_select(alang[1], 1, 2)
            diag_select(alang[1], 1, 3)
            vaug_chunk(10)
            vg_chunk(10)
            vaug_chunk(11)
            vg_chunk(11)

            # pos1 grav scores; fillers: last vaug chunks, remaining pos0
            # A@v, then the pos1 lang A@v chains. Diag pairs first.
            score_pair(agrav[1], yT, 12, P1, yq1, SC_GRAV, 10)
            diag_select(agrav[1], 1, 0)
            diag_select(agrav[1], 1, 1)
            vaug_chunk(12)
            vg_chunk(12)
            vaug_chunk(13)
            vg_chunk(13)
            score_pair(agrav[1], yT, 14, P1, yq1, SC_GRAV, 11)
            diag_select(agrav[1], 1, 2)
            diag_select(agrav[1], 1, 3)
            vaug_chunk(14)
            vg_chunk(14)
            vaug_chunk(15)
            vg_chunk(15)
            psb = [None] * 4
            score_pair(agrav[1], yT, 0, P1, yq1, SC_GRAV, 4)
            attn_j(0, 2)
            score_pair(agrav[1], yT, 2, P1, yq1, SC_GRAV, 5)
            psb[3] = attn_pol(1, 3)
            score_pair(agrav[1], yT, 4, P1, yq1, SC_GRAV, 6)
            psb[2] = attn_pol(1, 2)
            score_pair(agrav[1], yT, 6, P1, yq1, SC_GRAV, 7)
            psb[1] = attn_pol(1, 1)
            score_pair(agrav[1], yT, 8, P1, yq1, SC_GRAV, 8)
            psb[0] = attn_pol(1, 0)
            score_pair(agrav[1], yT, 10, P1, yq1, SC_GRAV, 9)
            attn_j(0, 3)

            # pos1 grav A@v + blends (longest chain first, shortest last)
            for j in (3, 2, 1, 0):
                attn_pog(1, j, psb[j])

    nc.finalize()
    return nc


def _host_inputs(x, Wq, Wk, Wv, L_grav):
    """Build the 8 per-core input maps (permuted key layout per half)."""
    f16 = np.float16
    x = np.asarray(x, np.float32)
    Wq = np.asarray(Wq, np.float32)
    Wk = np.asarray(Wk, np.float32)
    Wv = np.asarray(Wv, np.float32)
    L = np.asarray(L_grav, np.float32)

    def warr(w):  # [1024,128] -> [128, 8*128] chunk-major for lhsT slices
        return np.ascontiguousarray(
            w.reshape(8, 128, 128).transpose(1, 0, 2).reshape(128, 8 * 128)
        ).astype(f16)

    wqa, wka, wva = warr(Wq), warr(Wk), warr(Wv)
    lga = L.astype(f16)

    def half_bf(h):
        """bf [128, 12] f32: per-pair exp bias, 0 = keep, KILL = drop.

        cols 0-3: pos0 pairs (slots 0-1,2-3,4-5,6-7);
        cols 4-11: pos1 pairs (slots 8-9 ... 14-15 of the pos1 loop).
        """
        groups = GROUPS[h]
        v = np.zeros(12, np.float32)
        # pos0 pairs 0,1 = chunks 4-7 (group slot 1) vs pos0 queries (slot 0)
        if groups[1] > groups[0]:
            v[0] = v[1] = KILL
        # pos1 pairs 4,5 = chunks 12-15 (group slot 3) vs pos1 queries (slot 2)
        if groups[3] > groups[2]:
            v[8] = v[9] = KILL
        return np.ascontiguousarray(np.broadcast_to(v, (128, 12))).copy()

    bfs = [half_bf(0), half_bf(1)]
    in_maps = []
    for core in range(8):
        b, h = core // 2, core % 2
        xTb = x[b].T.astype(f16)  # [1024, 2048]
        xp = np.concatenate([xTb[:, g * 512:(g + 1) * 512] for g in GROUPS[h]],
                            axis=1)
        in_maps.append({
            "xT": np.ascontiguousarray(xp),
            "wq": wqa, "wk": wka, "wv": wva, "lg": lga,
            "bf": bfs[h],
        })
    return in_maps


def kernel(x, Wq, Wk, Wv, L_grav):
    import concourse.bass_utils as bass_utils

    if "nc" not in _CACHE:
        _CACHE["nc"] = _build_nc()
    nc = _CACHE["nc"]
    in_maps = _host_inputs(x, Wq, Wk, Wv, L_grav)

    trace = bool(os.environ.get("BASS_KERNEL_TRACE"))
    if trace:
        bass_utils.upload_artifacts = lambda tmpdir: f"file://{tmpdir}"
    res = bass_utils.run_bass_kernel_spmd(nc, in_maps, list(range(8)), trace=trace)
    if trace:
        _CACHE["exec_time_ns"] = res.exec_time_ns
        _CACHE["mean_exec_time_ns"] = res.mean_exec_time_ns

    out = np.empty((B, N, D_HEAD), np.float32)
    for core in range(8):
        b, h = core // 2, core % 2
        r = np.asarray(res.results[core]["out"], np.float32)
        g0, g2 = GROUPS[h][0], GROUPS[h][2]
        out[b, g0 * 512:(g0 + 1) * 512] = r[0:512]
        out[b, g2 * 512:(g2 + 1) * 512] = r[512:1024]
    return out


# revision 27
# speedup vs baseline: 1.0300x; 1.0300x over previous
"""Bass/Trainium2 kernel for nn_Head_13030930776875.

out = 0.7*softmax(causal(x@Wq @ (x@Wk)^T / sqrt(d))) @ (x@Wv)
    + 0.3*rownorm(causal(exp(-|y_i - y_j|^2 / (2d)))) @ (x@Wv),  y = (x@Wk)@L_grav

Sharding: 8 cores = 4 samples x 2 halves. Each half owns two 512-row query
groups chosen so causal (triangular) work balances: half0 -> {G0, G3},
half1 -> {G1, G2}.

Permuted key layout: the host ships x^T with its 512-column groups permuted
per half (h0: [G0,G1,G3,G2], h1: [G1,G0,G2,G3]) so that SBUF group 0 is the
pos0 query group and group 2 is the pos1 query group FOR EVERY CORE. The
program is SPMD-uniform; causality over the permuted key order is data-
driven via a tiny per-pair exp-bias table (0 / -40 kills whole chunks) plus
compile-time affine_select band masks for the diagonal chunks (which sit at
the same slot positions on every core).

On-device layout: everything transposed (d on partitions). Scores are
computed as s^T tiles [k,q] so that (a) A^T slices feed the A@v matmul
directly as the stationary operand (no transposes anywhere), and (b) the
causal row-sums come free via ones-columns appended to v.

Score exps are batched: two 512-wide score matmuls share one [128,1024]
PSUM tile and a single exp ACT whose per-partition bias AP kills fully-
invalid chunk pairs. The grav kernel factorizes as exp(gram/128)*ek with
ek = exp(-sq_k/256) folded into a second v copy (vg = v*ek) so no per-slot
mask multiplies are needed anywhere; diagonal bands are zeroed in-place by
gpsimd affine_select.
"""

import math
import os

import numpy as np

B, N, D_MODEL, D_HEAD = 4, 2048, 1024, 128
OMEGA_LANG, OMEGA_GRAV = 0.7, 0.3
SC_LANG = 1.0 / math.sqrt(D_HEAD)
SC_GRAV = 1.0 / D_HEAD
NBLK = N // 128            # 16 k-chunks of 128
NCH = (8, 16)              # chunks per position (pos0 group, pos1 group)
NSLOT = 16                 # mask slots: pos0 loop 0-7, pos1 loop 8-15
KILL = -40.0               # exp bias for fully-invalid chunk pairs
# chunk processing order per position (band group last so the A@v prefix
# structure nkb = 5+j / 13+j covers exactly the causally-needed chunks)
P0 = [4, 5, 6, 7, 0, 1, 2, 3]
P1 = [0, 1, 2, 3, 4, 5, 6, 7, 12, 13, 14, 15, 8, 9, 10, 11]
# original group id per SBUF group slot, per half
GROUPS = [[0, 1, 3, 2], [1, 0, 2, 3]]

_CACHE = {}


def _build_nc():
    import concourse.bacc as bacc
    import concourse.mybir as mybir
    import concourse.tile as tile
    import concourse.bass as bass

    dt = mybir.dt
    F16, F32 = dt.float16, dt.float32
    AF = mybir.ActivationFunctionType
    OP = mybir.AluOpType

    nc = bacc.Bacc()

    xT = nc.declare_dram_parameter("xT", [D_MODEL, N], F16, isOutput=False)
    wq = nc.declare_dram_parameter("wq", [128, 8 * 128], F16, isOutput=False)
    wk = nc.declare_dram_parameter("wk", [128, 8 * 128], F16, isOutput=False)
    wv = nc.declare_dram_parameter("wv", [128, 8 * 128], F16, isOutput=False)
    lg = nc.declare_dram_parameter("lg", [128, 128], F16, isOutput=False)
    bf = nc.declare_dram_parameter("bf", [128, 12], F32, isOutput=False)
    out_d = nc.declare_dram_parameter("out", [N // 2, 128], F16, isOutput=True)

    with tile.TileContext(nc) as tc:
        with (
            tc.tile_pool(name="big", bufs=1) as big,
            tc.tile_pool(name="xtp", bufs=8) as xtp,
            tc.tile_pool(name="ap0", bufs=1) as ap0,
            tc.tile_pool(name="ap1", bufs=1) as ap1,
            tc.tile_pool(name="small", bufs=4) as small,
            tc.tile_pool(name="pols", bufs=4) as pols,
            tc.tile_pool(name="outp", bufs=4) as outp,
            tc.tile_pool(name="score", bufs=2, space="PSUM") as score,
            tc.tile_pool(name="pp", bufs=2, space="PSUM") as pp,
            tc.tile_pool(name="av", bufs=2, space="PSUM") as av,
        ):
            # ---- small inputs on the scalar DGE ring (parallel to x) ----
            wq_s = big.tile([128, 8, 128], F16, tag="wq")
            wk0_s = big.tile([128, 128], F16, tag="wk0")
            wkr_s = big.tile([128, 7, 128], F16, tag="wkr")
            wv_s = big.tile([128, 8, 128], F16, tag="wv")
            lg_s = big.tile([128, 128], F16, tag="lg")
            bf_s = big.tile([128, 12], F32, tag="bf")
            nc.scalar.dma_start(wk0_s[:], wk[:, 0:128])
            nc.scalar.dma_start(wkr_s[:],
                                wk[:, 128:1024].rearrange("p (c d) -> p c d", c=7))
            nc.scalar.dma_start(lg_s[:], lg[:])
            nc.scalar.dma_start(wq_s[:], wq[:].rearrange("p (c d) -> p c d", c=8))
            nc.scalar.dma_start(wv_s[:], wv[:].rearrange("p (c d) -> p c d", c=8))
            nc.scalar.dma_start(bf_s[:], bf[:])

            # ---- x^T in 1024-col halves (2KB DMA lines), need-ordered ----
            xt = [xtp.tile([128, N], F16, tag="xt", name=f"xt{c}")
                  for c in range(8)]
            nc.sync.dma_start(xt[0][:, 0:512], xT[0:128, 0:512])
            nc.sync.dma_start(xt[0][:, 512:1024], xT[0:128, 512:1024])
            for c in range(1, 8):
                nc.sync.dma_start(xt[c][:, 0:1024], xT[c * 128:(c + 1) * 128, 0:1024])
            for c in range(8):
                nc.sync.dma_start(xt[c][:, 1024:2048],
                                  xT[c * 128:(c + 1) * 128, 1024:2048])

            # ---- persistent intermediates ----
            kT = big.tile([128, N], F16, tag="kT")
            qT = big.tile([128, N // 2], F16, tag="qT")
            yT = big.tile([128, N], F16, tag="yT")
            sqn = big.tile([128, NBLK], F32, tag="sqn")
            ek_s = big.tile([128, NBLK], F32, tag="ek")
            vaug = big.tile([128, NBLK, 130], F16, tag="vaug")
            vg = big.tile([128, NBLK, 130], F16, tag="vg")
            nc.vector.memset(vaug[:, :, 128:129], 1.0 / OMEGA_LANG)
            nc.vector.memset(vaug[:, :, 129:130], 1.0 / OMEGA_GRAV)

            def wk_c(c):
                return wk0_s[:] if c == 0 else wkr_s[:, c - 1, :]

            def proj_group(dst, dcol, w_sb, g):
                cols = slice(g * 512, (g + 1) * 512)
                ps = pp.tile([128, 512], F32, tag="pp")
                for c in range(8):
                    w = wk_c(c) if w_sb is None else w_sb[:, c, :]
                    nc.tensor.matmul(ps[:], w, xt[c][:, cols],
                                     start=(c == 0), stop=(c == 7))
                nc.vector.tensor_copy(dst[:, dcol * 512:(dcol + 1) * 512], ps[:])

            def yt_group(g):
                cols = slice(g * 512, (g + 1) * 512)
                ps = pp.tile([128, 512], F32, tag="pp")
                nc.tensor.matmul(ps[:], lg_s[:], kT[:, cols])
                nc.vector.tensor_copy(yT[:, cols], ps[:])

            def sqn_quad(q4):
                # y chunks for 4 key blocks -> [128,4,128] psum; squared on
                # ACT ((y/16)^2 = y^2/256), then one DVE reduce -> sq/256.
                ps = pp.tile([128, 4, 128], F32, tag="pp")
                for i in range(4):
                    kb = q4 * 4 + i
                    nc.tensor.matmul(ps[:, i, :], kT[:, kb * 128:(kb + 1) * 128],
                                     lg_s[:])
                scr = small.tile([128, 4, 128], F32, tag="scr")
                nc.scalar.activation(scr[:], ps[:], AF.Square, scale=0.0625)
                nc.vector.tensor_reduce(sqn[:, q4 * 4:(q4 + 1) * 4], scr[:],
                                        mybir.AxisListType.X, OP.add)

            def vaug_chunk(kb):
                ps = pp.tile([128, 512], F32, tag="pp")
                for c in range(8):
                    nc.tensor.matmul(ps[:, 0:128], xt[c][:, kb * 128:(kb + 1) * 128],
                                     wv_s[:, c, :], start=(c == 0), stop=(c == 7))
                nc.vector.tensor_copy(vaug[:, kb, 0:128], ps[:, 0:128])

            def vg_chunk(kb):
                # grav v copy with ek = exp(-sq_k/256) folded in; col 129
                # becomes ek/OMEGA_GRAV (the grav rowsum weight).
                nc.vector.tensor_scalar(vg[:, kb, :], vaug[:, kb, :],
                                        ek_s[:, kb:kb + 1], None, OP.mult)

            def diag_select(t, pos, i):
                # zero the causally-invalid band of own-group chunk i
                # (slot 4+i for pos0, 12+i for pos1): valid iff q >= p + i*128
                s = (4 if pos == 0 else 12) + i
                nc.gpsimd.affine_select(
                    t[:, s * 512:(s + 1) * 512], t[:, s * 512:(s + 1) * 512],
                    pattern=[[1, 512]], compare_op=OP.is_ge, fill=0.0,
                    base=-i * 128, channel_multiplier=-1)

            # PE warmup woven into the first kT projection group: 2 dummy
            # matmuls per x chunk, each consuming only already-arrived data,
            # so the PE tracks DMA arrival and the HAM clock-gate opens
            # (4/8 -> 8/8) by the time the dense stream begins.
            def kt_g0_with_warmup():
                warm = pp.tile([128, 512], F32, tag="pp")
                ps = pp.tile([128, 512], F32, tag="pp")
                for c in range(8):
                    for i in range(2):
                        nc.tensor.matmul(warm[:], xt[c][:, 0:128],
                                         xt[c][:, 0:512],
                                         start=(c == 0 and i == 0),
                                         stop=(c == 7 and i == 1))
                    nc.tensor.matmul(ps[:], wk_s[:, c, :], xt[c][:, 0:512],
                                     start=(c == 0), stop=(c == 7))
                nc.vector.tensor_copy(kT[:, 0:512], ps[:])

            def score_pair(dst, lhs, slot, plist, qrhs, scale, bfc):
                # two 512-wide score matmuls (chunks plist[slot], plist[slot+1])
                # -> one [128,1024] psum tile, one exp ACT whose per-partition
                # bias (0 / KILL) drops fully-invalid chunk pairs.
                ca, cb = plist[slot], plist[slot + 1]
                ps = score.tile([128, 1024], F32, tag="sc")
                nc.tensor.matmul(ps[:, 0:512], lhs[:, ca * 128:(ca + 1) * 128], qrhs)
                nc.tensor.matmul(ps[:, 512:1024], lhs[:, cb * 128:(cb + 1) * 128],
                                 qrhs)
                nc.scalar.activation(dst[:, slot * 512:(slot + 2) * 512], ps[:],
                                     AF.Exp, scale=scale,
                                     bias=bf_s[:, bfc:bfc + 1])

            def attn_j(pos, j):
                # A^T @ v_aug for one 128-row query block; rowsums ride cols
                # 128 (lang, 1/0.7) and 129 (grav, ek/0.3). The grav chain
                # accumulates chunks whose exp lands latest at the end so the
                # PE can start the chain before the last grav ACTs drain.
                nkb = (5 + j) if pos == 0 else (13 + j)
                plist = P0 if pos == 0 else P1
                ks = list(range(nkb))
                gks = ([kb for kb in ks if kb >= 12] +
                       [kb for kb in ks if kb < 8] +
                       [kb for kb in ks if 8 <= kb < 12]) if pos == 1 else ks
                pol = av.tile([128, 132], F32, tag="av")
                pog = av.tile([128, 132], F32, tag="av")
                for kb in ks:
                    nc.tensor.matmul(pol[:, 0:129],
                                     alang[pos][:, kb * 512 + j * 128:kb * 512 + (j + 1) * 128],
                                     vaug[:, plist[kb], 0:129],
                                     start=(kb == ks[0]), stop=(kb == ks[-1]))
                for kb in gks:
                    nc.tensor.matmul(pog[:, 0:130],
                                     agrav[pos][:, kb * 512 + j * 128:kb * 512 + (j + 1) * 128],
                                     vg[:, plist[kb], 0:130],
                                     start=(kb == gks[0]), stop=(kb == gks[-1]))
                blend_out(pos, j, pol, pog)

            def blend_out(pos, j, pol, pog):
                rl = small.tile([128, 1], F32, tag="rl")
                rg = small.tile([128, 1], F32, tag="rg")
                nc.vector.reciprocal(rl[:], pol[:, 128:129])
                nc.vector.reciprocal(rg[:], pog[:, 129:130])
                ob = outp.tile([128, 128], F32, tag="ob")
                ob2 = outp.tile([128, 128], F16, tag="ob2")
                nc.vector.tensor_scalar(ob[:], pol[:, 0:128], rl[:], None, OP.mult)
                nc.vector.scalar_tensor_tensor(ob2[:], pog[:, 0:128], rg[:], ob[:],
                                               OP.mult, OP.add)
                r0 = pos * 512 + j * 128
                nc.sync.dma_start(out_d[r0:r0 + 128, :], ob2[:])

            def attn_pol(pos, j):
                # lang-only A@v chain (no ACT dependency); numerator+rowsum
                # parked in SBUF so the PSUM bank recycles under the
                # score-exp phase still in flight.
                nkb = (5 + j) if pos == 0 else (13 + j)
                plist = P0 if pos == 0 else P1
                pol = av.tile([128, 132], F32, tag="av")
                for kb in range(nkb):
                    nc.tensor.matmul(pol[:, 0:129],
                                     alang[pos][:, kb * 512 + j * 128:kb * 512 + (j + 1) * 128],
                                     vaug[:, plist[kb], 0:129],
                                     start=(kb == 0), stop=(kb == nkb - 1))
                psb = pols.tile([128, 132], F32, tag="polsb")
                nc.vector.tensor_copy(psb[:, 0:129], pol[:, 0:129])
                rl = small.tile([128, 1], F32, tag="rl")
                nc.vector.reciprocal(rl[:], pol[:, 128:129])
                ob = outp.tile([128, 128], F32, tag="obp")
                nc.vector.tensor_scalar(ob[:], pol[:, 0:128], rl[:], None, OP.mult)
                return (psb, ob)

            def attn_pog(pos, j, psb_ob):
                psb, ob = psb_ob
                nkb = (5 + j) if pos == 0 else (13 + j)
                plist = P0 if pos == 0 else P1
                ks = list(range(nkb))
                gks = ([kb for kb in ks if kb >= 12] +
                       [kb for kb in ks if kb < 8] +
                       [kb for kb in ks if 8 <= kb < 12]) if pos == 1 else ks
                pog = av.tile([128, 132], F32, tag="av")
                for kb in gks:
                    nc.tensor.matmul(pog[:, 0:130],
                                     agrav[pos][:, kb * 512 + j * 128:kb * 512 + (j + 1) * 128],
                                     vg[:, plist[kb], 0:130],
                                     start=(kb == gks[0]), stop=(kb == gks[-1]))
                rg = small.tile([128, 1], F32, tag="rg")
                nc.vector.reciprocal(rg[:], pog[:, 129:130])
                ob2 = outp.tile([128, 128], F16, tag="ob2")
                nc.vector.scalar_tensor_tensor(ob2[:], pog[:, 0:128], rg[:], ob[:],
                                               OP.mult, OP.add)
                r0 = pos * 512 + j * 128
                nc.scalar.dma_start(out_d[r0:r0 + 128, :], ob2[:])

            # ================= h0 stream, tracked: kT groups 0+1 accumulate
            # chunk-by-chunk as the first x halves land. The 2 MMs/chunk at
            # cold clock (~850ns) match the ~740ns chunk arrival cadence, so
            # this doubles as the HAM warmup with zero dummy work.
            ps_k0 = pp.tile([128, 512], F32, tag="pp")
            ps_k1 = pp.tile([128, 512], F32, tag="pp")
            for c in range(8):
                nc.tensor.matmul(ps_k0[:], wk_c(c), xt[c][:, 0:512],
                                 start=(c == 0), stop=(c == 7))
                nc.tensor.matmul(ps_k1[:], wk_c(c), xt[c][:, 512:1024],
                                 start=(c == 0), stop=(c == 7))
            nc.vector.tensor_copy(kT[:, 0:512], ps_k0[:])
            nc.vector.tensor_copy(kT[:, 512:1024], ps_k1[:])
            proj_group(qT, 0, wq_s, 0)
            q0 = qT[:, 0:512]
            yq0 = yT[:, 0:512]
            yt_group(0)
            yt_group(1)

            alang = [None, None]
            agrav = [None, None]
            alang[0] = ap0.tile([128, NCH[0] * 512], F16, tag="al0", name="al0")
            agrav[0] = ap0.tile([128, NCH[0] * 512], F16, tag="ag0", name="ag0")

            # pos0 lang scores; fillers: sqn quads + the tracked kT g2 /
            # qT g1 projections that consume the h1 halves as they land
            score_pair(alang[0], kT, 0, P0, q0, SC_LANG, 0)
            sqn_quad(0)
            score_pair(alang[0], kT, 2, P0, q0, SC_LANG, 1)
            sqn_quad(1)
            score_pair(alang[0], kT, 4, P0, q0, SC_LANG, 2)
            nc.scalar.activation(ek_s[:, 0:8], sqn[:, 0:8], AF.Exp, scale=-1.0)
            diag_select(alang[0], 0, 0)
            diag_select(alang[0], 0, 1)
            score_pair(alang[0], kT, 6, P0, q0, SC_LANG, 3)
            diag_select(alang[0], 0, 2)
            diag_select(alang[0], 0, 3)

            # pos0 grav scores; fillers: vaug chunks 0-7 (+ vg) and the
            # remaining tracked h1 projections
            for p in range(4):
                score_pair(agrav[0], yT, 2 * p, P0, yq0, SC_GRAV, p)
                if p >= 2:
                    diag_select(agrav[0], 0, 2 * p - 4)
                    diag_select(agrav[0], 0, 2 * p - 3)
                vaug_chunk(2 * p)
                vg_chunk(2 * p)
                vaug_chunk(2 * p + 1)
                vg_chunk(2 * p + 1)
            # by now all second x halves have landed: qT g1 runs stall-free
            proj_group(qT, 1, wq_s, 2)
            q1 = qT[:, 512:1024]
            yq1 = yT[:, 1024:1536]

            # ================= pos1 lang scores; fillers: pos0 A@v chains,
            # kT g3, yt/sqn for the upper groups, vaug chunks 8-15
            alang[1] = ap1.tile([128, NCH[1] * 512], F16, tag="al1", name="al1")
            agrav[1] = ap1.tile([128, NCH[1] * 512], F16, tag="ag1", name="ag1")
            score_pair(alang[1], kT, 0, P1, q1, SC_LANG, 4)
            attn_j(0, 0)
            score_pair(alang[1], kT, 2, P1, q1, SC_LANG, 5)
            attn_j(0, 1)
            score_pair(alang[1], kT, 4, P1, q1, SC_LANG, 6)
            proj_group(kT, 3, None, 3)
            score_pair(alang[1], kT, 6, P1, q1, SC_LANG, 7)
            proj_group(kT, 2, None, 2)
            score_pair(alang[1], kT, 8, P1, q1, SC_LANG, 8)
            yt_group(2)
            sqn_quad(2)
            score_pair(alang[1], kT, 10, P1, q1, SC_LANG, 9)
            yt_group(3)
            sqn_quad(3)
            nc.scalar.activation(ek_s[:, 8:16], sqn[:, 8:16], AF.Exp,
                                 scale=-1.0)
            score_pair(alang[1], kT, 12, P1, q1, SC_LANG, 10)
            diag_select(alang[1], 1, 0)
            diag_select(alang[1], 1, 1)
            vaug_chunk(8)
            vg_chunk(8)
            vaug_chunk(9)
            vg_chunk(9)
            score_pair(alang[1], kT, 14, P1, q1, SC_LANG, 11)
            diag_select(alang[1], 1, 2)
            diag_select(alang[1], 1, 3)
            vaug_chunk(10)
            vg_chunk(10)
            vaug_chunk(11)
            vg_chunk(11)

            # pos1 grav scores; fillers: last vaug chunks, remaining pos0
            # A@v, then the pos1 lang A@v chains. Diag pairs first.
            score_pair(agrav[1], yT, 12, P1, yq1, SC_GRAV, 10)
            diag_select(agrav[1], 1, 0)
            diag_select(agrav[1], 1, 1)
            vaug_chunk(12)
            vg_chunk(12)
            vaug_chunk(13)
            vg_chunk(13)
            score_pair(agrav[1], yT, 14, P1, yq1, SC_GRAV, 11)
            diag_select(agrav[1], 1, 2)
            diag_select(agrav[1], 1, 3)
            vaug_chunk(14)
            vg_chunk(14)
            vaug_chunk(15)
            vg_chunk(15)
            psb = [None] * 4
            score_pair(agrav[1], yT, 0, P1, yq1, SC_GRAV, 4)
            attn_j(0, 2)
            score_pair(agrav[1], yT, 2, P1, yq1, SC_GRAV, 5)
            psb[3] = attn_pol(1, 3)
            score_pair(agrav[1], yT, 4, P1, yq1, SC_GRAV, 6)
            psb[2] = attn_pol(1, 2)
            score_pair(agrav[1], yT, 6, P1, yq1, SC_GRAV, 7)
            psb[1] = attn_pol(1, 1)
            score_pair(agrav[1], yT, 8, P1, yq1, SC_GRAV, 8)
            psb[0] = attn_pol(1, 0)
            score_pair(agrav[1], yT, 10, P1, yq1, SC_GRAV, 9)
            attn_j(0, 3)

            # pos1 grav A@v + blends (longest chain first, shortest last)
            for j in (3, 2, 1, 0):
                attn_pog(1, j, psb[j])

    nc.finalize()
    return nc


def _host_inputs(x, Wq, Wk, Wv, L_grav):
    """Build the 8 per-core input maps (permuted key layout per half)."""
    f16 = np.float16
    x = np.asarray(x, np.float32)
    Wq = np.asarray(Wq, np.float32)
    Wk = np.asarray(Wk, np.float32)
    Wv = np.asarray(Wv, np.float32)
    L = np.asarray(L_grav, np.float32)

    def warr(w):  # [1024,128] -> [128, 8*128] chunk-major for lhsT slices
        return np.ascontiguousarray(
            w.reshape(8, 128, 128).transpose(1, 0, 2).reshape(128, 8 * 128)
        ).astype(f16)

    wqa, wka, wva = warr(Wq), warr(Wk), warr(Wv)
    lga = L.astype(f16)

    def half_bf(h):
        """bf [128, 12] f32: per-pair exp bias, 0 = keep, KILL = drop.

        cols 0-3: pos0 pairs (slots 0-1,2-3,4-5,6-7);
        cols 4-11: pos1 pairs (slots 8-9 ... 14-15 of the pos1 loop).
        """
        groups = GROUPS[h]
        v = np.zeros(12, np.float32)
        # pos0 pairs 0,1 = chunks 4-7 (group slot 1) vs pos0 queries (slot 0)
        if groups[1] > groups[0]:
            v[0] = v[1] = KILL
        # pos1 pairs 4,5 = chunks 12-15 (group slot 3) vs pos1 queries (slot 2)
        if groups[3] > groups[2]:
            v[8] = v[9] = KILL
        return np.ascontiguousarray(np.broadcast_to(v, (128, 12))).copy()

    bfs = [half_bf(0), half_bf(1)]
    in_maps = []
    for core in range(8):
        b, h = core // 2, core % 2
        xTb = x[b].T.astype(f16)  # [1024, 2048]
        xp = np.concatenate([xTb[:, g * 512:(g + 1) * 512] for g in GROUPS[h]],
                            axis=1)
        in_maps.append({
            "xT": np.ascontiguousarray(xp),
            "wq": wqa, "wk": wka, "wv": wva, "lg": lga,
            "bf": bfs[h],
        })
    return in_maps


def kernel(x, Wq, Wk, Wv, L_grav):
    import concourse.bass_utils as bass_utils

    if "nc" not in _CACHE:
        _CACHE["nc"] = _build_nc()
    nc = _CACHE["nc"]
    in_maps = _host_inputs(x, Wq, Wk, Wv, L_grav)

    trace = bool(os.environ.get("BASS_KERNEL_TRACE"))
    if trace:
        bass_utils.upload_artifacts = lambda tmpdir: f"file://{tmpdir}"
    res = bass_utils.run_bass_kernel_spmd(nc, in_maps, list(range(8)), trace=trace)
    if trace:
        _CACHE["exec_time_ns"] = res.exec_time_ns
        _CACHE["mean_exec_time_ns"] = res.mean_exec_time_ns

    out = np.empty((B, N, D_HEAD), np.float32)
    for core in range(8):
        b, h = core // 2, core % 2
        r = np.asarray(res.results[core]["out"], np.float32)
        g0, g2 = GROUPS[h][0], GROUPS[h][2]
        out[b, g0 * 512:(g0 + 1) * 512] = r[0:512]
        out[b, g2 * 512:(g2 + 1) * 512] = r[512:1024]
    return out
